# revision 25
# baseline (speedup 1.0000x reference)
"""Trainium2 Bass kernel for the spiking-actor MLP (nn_Actor_3504693313593).

Math (per batch element, T=16 steps, LIF tau=2, v_th=1, hard reset to 0):
  h1 = state @ W1.T + b1                      (loop invariant)
  per step: v1 = (v1 + h1)/2; s1 = v1>=1; v1 *= 1-s1
            h2 = s1 @ W2.T + b2; v2 = (v2 + h2)/2; s2 = v2>=1; v2 *= 1-s2
            v3 = (v3 + s2 @ W3.T + b3)/2      (non-spiking, linear)
  out = tanh(v3)

Device formulation (validated exact vs reference):
  - spikes carried as g = 2s-1 in {-1,+1} (ScalarE Sign), the +1 halves folded
    into weights/biases on the host.
  - per-layer LIF state kept TIME-SCALED and BIAS-SHIFTED:  Wt = 2^t (u_t - b)
    so one fused custom-DVE op per layer per step does charge+reset+bias:
        Wt' = select(Wt < 2^(t-1)(1-b), Wt + 2^(t-1) b, 0) + mm * 2^t
    (for layer 1 the bias lives inside h1h, so the C1 slot is 0).
  - spikes: g = Sign(2^-t * Wt + (b-1)) on ScalarE with per-partition bias.
  - layer 3 is linear in s2 -> collapsed into a PSUM accumulation over steps
    with host-prescaled weights 2^(t-18)*W3, hi/lo packed along M (16 rows),
    4 batch-chunks at PSUM col-groups 0..3 of a single bank.
  - matmuls in bf16 with hi/lo splitting (exact to 2^-17); spikes exact bf16.
Sharding: pure data parallel, batch/8 per NeuronCore, weights replicated.
"""

import numpy as np

S, H, A, T = 128, 256, 8, 16
NCORES = 8
B_GLOBAL = 131072
B_C = B_GLOBAL // NCORES   # 16384 rows per core
N_B = 2048                 # batch-tile (SBUF free dim)
NT = B_C // N_B            # 8 batch tiles per core
CH = 512                   # matmul moving-dim chunk
NCH = N_B // CH            # 4 chunks per tile
HF = N_B // 2              # 1024: psum tile free dim (2 banks)
NBLK = N_B // 128          # 16 128-row blocks per tile

_CACHE = {}


def _register_lif_op():
    """Fused LIF step: out = select(in1 < s0, in1 + s1, 0) + in0*imm2."""
    import concourse.dve_ops as dve_ops
    if "LIF_STEP_ANT" in dve_ops._SUB_OPCODE_FOR_NAME:
        return next(o for o in dve_ops.OPS if o.name == "LIF_STEP_ANT")
    from concourse.dve_spec import Spec, Src0, Src1, C0, C1, C2, Zero, select, lower
    from concourse.dve_uop import DveOpSpec

    body = select(Src1 < C0, Src1 + C1, Zero) + Src0 * C2
    spec = Spec(
        body=body,
        reference=lambda in0, in1, s0, s1, imm2: (
            np.where(in1 < s0, in1 + s1, 0.0) + in0.astype(np.float32) * imm2
        ).astype(np.float32),
    )
    row = max(dve_ops._SUB_OPCODE_FOR_NAME.values()) + 1
    dve_ops._SUB_OPCODE_FOR_NAME["LIF_STEP_ANT"] = row
    shas = {}
    for ver in ("v3",):
        tmp = DveOpSpec(name="LIF_STEP_ANT", opcode=row,
                        uops=lower(spec, ver=ver), rd1_en=True)
        shas[ver] = tmp.sha(ver)
    op = dve_ops.DveOp("LIF_STEP_ANT", spec, subdim=False, uops_sha=shas)
    dve_ops.OPS.append(op)
    dve_ops.CUSTOM_DVE_SPECS[op.name] = spec
    return op


def _build_nc():
    import concourse.bass as bass
    import concourse.bacc as bacc
    import concourse.mybir as mybir

    f32 = mybir.dt.float32
    bf16 = mybir.dt.bfloat16
    AF = mybir.ActivationFunctionType
    OP = mybir.AluOpType
    from concourse.tile import TileContext

    LIF = _register_lif_op()

    nc = bacc.Bacc()
    state_d = nc.declare_dram_parameter("state", [B_C, S], f32, isOutput=False)
    w1_d = [nc.declare_dram_parameter(f"w1ht{h}", [S, H], bf16, isOutput=False)
            for h in range(2)]                                    # (W1/2).T hi/lo
    w2_d = [nc.declare_dram_parameter(f"w2qt{h}", [H, H], bf16, isOutput=False)
            for h in range(2)]                                    # (W2/4).T hi/lo
    w3_d = nc.declare_dram_parameter("w3pack", [H, T * 16], bf16, isOutput=False)
    b1_d = nc.declare_dram_parameter("b1h", [2, 128], f32, isOutput=False)
    bthr_d = nc.declare_dram_parameter("b2thr", [2, 128, T], f32, isOutput=False)
    badd_d = nc.declare_dram_parameter("b2add", [2, 128, T], f32, isOutput=False)
    bcol_d = nc.declare_dram_parameter("b2cols", [2, 128, 2], f32, isOutput=False)
    b3_d = nc.declare_dram_parameter("b3z", [16, 1], f32, isOutput=False)
    id_d = nc.declare_dram_parameter("ident", [128, 128], f32, isOutput=False)
    out_d = nc.declare_dram_parameter("out", [B_C, A], f32, isOutput=True)

    with TileContext(nc) as tc:
        with (
            tc.tile_pool(name="wp", bufs=1) as wp,
            tc.tile_pool(name="stin", bufs=2) as stin_p,
            tc.tile_pool(name="stT", bufs=2) as stT_p,
            tc.tile_pool(name="h1h", bufs=2) as h1h_p,
            tc.tile_pool(name="st8", bufs=1) as st_p,
            tc.tile_pool(name="y3", bufs=4) as y3_p,
            tc.tile_pool(name="outT", bufs=2) as outT_p,
            tc.tile_pool(name="ps", bufs=3, space="PSUM") as ps_p,
            tc.tile_pool(name="psv3", bufs=1, space="PSUM") as psv3_p,
        ):
            # ---- load weights/constants (once) ----
            w1t = [wp.tile([S, H], bf16, tag=f"w1t{h}", name=f"w1t{h}")
                   for h in range(2)]
            for h in range(2):
                nc.sync.dma_start(w1t[h][:], w1_d[h][:])
            w2t = [[wp.tile([128, H], bf16, tag=f"w2t{h}{k}", name=f"w2t{h}{k}")
                    for k in range(2)] for h in range(2)]
            for h in range(2):
                for k in range(2):
                    nc.sync.dma_start(w2t[h][k][:], w2_d[h][k * 128:(k + 1) * 128, :])
            w3t = [wp.tile([128, T * 16], bf16, tag=f"w3p{k}", name=f"w3p{k}")
                   for k in range(2)]
            for k in range(2):
                nc.sync.dma_start(w3t[k][:], w3_d[k * 128:(k + 1) * 128, :])
            b1t = wp.tile([128, 2], f32, tag="b1t", name="b1t")
            nc.sync.dma_start(b1t[:], b1_d[:].rearrange("m p -> p m"))
            bthr = [wp.tile([128, T], f32, tag=f"bthr{m}", name=f"bthr{m}")
                    for m in range(2)]
            badd = [wp.tile([128, T], f32, tag=f"badd{m}", name=f"badd{m}")
                    for m in range(2)]
            bcol = [wp.tile([128, 2], f32, tag=f"bcol{m}", name=f"bcol{m}")
                    for m in range(2)]
            for m in range(2):
                nc.sync.dma_start(bthr[m][:], bthr_d[m])
                nc.sync.dma_start(badd[m][:], badd_d[m])
                nc.sync.dma_start(bcol[m][:], bcol_d[m])
            b3t16 = wp.tile([16, 1], f32, tag="b3t16", name="b3t16")
            nc.sync.dma_start(b3t16[:], b3_d[:])
            idt = wp.tile([128, 128], f32, tag="idt", name="idt")
            nc.sync.dma_start(idt[:], id_d[:])
            neg1 = wp.tile([128, 1], f32, tag="neg1", name="neg1")
            nc.vector.memset(neg1[:], -1.0)

            def load_phase(bt):
                """DMA state tile, transpose to [S, N_B] hi/lo, h1h matmuls."""
                t0 = bt * N_B
                st_in = stin_p.tile([128, N_B], f32, tag="st_in", name="st_in")
                nc.sync.dma_start(
                    st_in[:].rearrange("p (c s) -> p c s", s=S),
                    state_d[t0:t0 + N_B, :].rearrange("(c p) s -> p c s", p=128),
                )
                stTh = stT_p.tile([S, N_B], bf16, tag="stTh", name="stTh")
                stTl = stT_p.tile([S, N_B], bf16, tag="stTl", name="stTl")
                for c in range(NBLK):
                    pst = ps_p.tile([128, 128], f32, tag="ps_mm", name="ps_tr")
                    nc.tensor.transpose(
                        pst[:], st_in[:, c * 128:(c + 1) * 128], idt[:])
                    sl = slice(c * 128, (c + 1) * 128)
                    nc.scalar.activation(stTh[:, sl], pst[:], AF.Copy)
                    nc.vector.tensor_tensor(
                        stTl[:, sl], pst[:], stTh[:, sl], OP.subtract)
                h1h = [h1h_p.tile([128, N_B], f32, tag=f"h1h{m}", name=f"h1h{m}")
                       for m in range(2)]
                for m in range(2):
                    msl = slice(m * 128, (m + 1) * 128)
                    for c in range(NCH):
                        psm = ps_p.tile([128, CH], f32, tag="ps_mm", name="ps_mm")
                        csl = slice(c * CH, (c + 1) * CH)
                        nc.tensor.matmul(
                            psm[:, :CH], w1t[0][:, msl], stTh[:, csl],
                            start=True, stop=False)
                        nc.tensor.matmul(
                            psm[:, :CH], w1t[0][:, msl], stTl[:, csl],
                            start=False, stop=False)
                        nc.tensor.matmul(
                            psm[:, :CH], w1t[1][:, msl], stTh[:, csl],
                            start=False, stop=True)
                        nc.scalar.activation(
                            h1h[m][:, csl], psm[:, :CH],
                            AF.Identity, bias=b1t[:, m:m + 1])
                return h1h

            h1h_next = load_phase(0)
            for bt in range(NT):
                t0 = bt * N_B
                h1h = h1h_next
                # ---- states (time-scaled, bias-shifted) + spike tiles ----
                u1 = [st_p.tile([128, N_B], f32, tag=f"u1_{m}", name=f"u1_{m}")
                      for m in range(2)]
                u2 = [st_p.tile([128, N_B], f32, tag=f"u2_{m}", name=f"u2_{m}")
                      for m in range(2)]
                g1 = [[st_p.tile([128, N_B], bf16, tag=f"g1_{d}{m}",
                                 name=f"g1_{d}{m}") for m in range(2)]
                      for d in range(2)]
                g2 = [[st_p.tile([128, N_B], bf16, tag=f"g2_{d}{m}",
                                 name=f"g2_{d}{m}") for m in range(2)]
                      for d in range(2)]
                for m in range(2):
                    nc.vector.memset(u1[m][:], 0.0)
                    # u2_0 = -b2q (bias-shifted state at t=0)
                    nc.vector.tensor_copy(
                        u2[m][:], bcol[m][:, 0:1].to_broadcast([128, N_B]))
                pv3 = psv3_p.tile([128, CH], f32, tag="pv3", name="pv3")

                for t in range(1, T + 1):
                    th = float(2.0 ** (t - 1))
                    sc = float(2.0 ** t)
                    # ---- layer 1: fused LIF (bias inside h1h) + Sign ----
                    for m in range(2):
                        nc.vector._custom_dve(
                            LIF, out=u1[m][:], in0=h1h[m][:], in1=u1[m][:],
                            s0=th, s1=0.0, imm2=sc)
                    for m in range(2):
                        for j in range(2):
                            jsl = slice(j * HF, (j + 1) * HF)
                            nc.scalar.activation(
                                g1[t % 2][m][:, jsl], u1[m][:, jsl], AF.Sign,
                                bias=neg1[:, 0:1], scale=1.0 / sc)
                    # ---- layer 2: mm in psum, fused LIF from psum + Sign ----
                    for m in range(2):
                        msl = slice(m * 128, (m + 1) * 128)
                        pj = [ps_p.tile([128, HF], f32, tag="ps_mm", name="pmm")
                              for _ in range(2)]
                        for hk, (hh, kk) in enumerate(
                                ((0, 0), (0, 1), (1, 0), (1, 1))):
                            for j in range(2):
                                for cc in range(2):
                                    csl = slice((j * 2 + cc) * CH,
                                                (j * 2 + cc + 1) * CH)
                                    nc.tensor.matmul(
                                        pj[j][:, cc * CH:(cc + 1) * CH],
                                        w2t[hh][kk][:, msl],
                                        g1[t % 2][kk][:, csl],
                                        start=(hk == 0), stop=(hk == 3))
                        for j in range(2):
                            jsl = slice(j * HF, (j + 1) * HF)
                            nc.vector._custom_dve(
                                LIF, out=u2[m][:, jsl], in0=pj[j][:],
                                in1=u2[m][:, jsl],
                                s0=bthr[m][:, t - 1:t], s1=badd[m][:, t - 1:t],
                                imm2=sc)
                    for m in range(2):
                        for j in range(2):
                            jsl = slice(j * HF, (j + 1) * HF)
                            nc.scalar.activation(
                                g2[t % 2][m][:, jsl], u2[m][:, jsl], AF.Sign,
                                bias=bcol[m][:, 1:2], scale=1.0 / sc)
                    # ---- layer 3: pv3[32c:32c+16] += W3pack_t @ g2 chunks ----
                    for c in range(NCH):
                        for k in range(2):
                            nc.tensor.matmul(
                                pv3[32 * c:32 * c + 16, :],
                                w3t[k][:, (t - 1) * 16:t * 16],
                                g2[t % 2][k][:, c * CH:(c + 1) * CH],
                                start=(t == 1 and k == 0),
                                stop=(t == T and k == 1),
                                tile_position=(0, 32 * c))
                    # prefetch next tile mid-loop so its load/h1 phase fills
                    # engine idle slots instead of stalling the boundary
                    if t == 6 and bt + 1 < NT:
                        h1h_next = load_phase(bt + 1)

                # ---- v3: evac [16,512] (+b3z on hi rows), transpose
                #      [16,128] blocks -> [128,16], hi+lo add on free dim,
                #      tanh, DMA out ----
                y3 = [y3_p.tile([16, CH], f32, tag=f"y3_{c}", name=f"y3_{c}")
                      for c in range(NCH)]
                for c in range(NCH):
                    nc.scalar.activation(
                        y3[c][:], pv3[32 * c:32 * c + 16, :], AF.Identity,
                        bias=b3t16[:, 0:1])
                outT = outT_p.tile([128, NBLK * A], f32, tag="outT", name="outT")
                for c in range(NCH):
                    for cc in range(4):
                        q = c * 4 + cc
                        pso = ps_p.tile([128, 16], f32, tag="ps_mm", name="ps_out")
                        nc.tensor.transpose(
                            pso[:, :16], y3[c][:, cc * 128:(cc + 1) * 128],
                            idt[:16, :16])
                        ev16 = y3_p.tile([128, 16], f32, tag="ev16", name="ev16")
                        nc.scalar.activation(ev16[:], pso[:, :16], AF.Copy)
                        nc.vector.tensor_tensor(
                            outT[:, q * A:(q + 1) * A],
                            ev16[:, 0:A], ev16[:, A:16], OP.add)
                nc.scalar.activation(outT[:], outT[:], AF.Tanh)
                nc.sync.dma_start(
                    out_d[t0:t0 + N_B, :].rearrange("(q p) a -> p q a", p=128),
                    outT[:].rearrange("p (q a) -> p q a", a=A),
                )
    return nc


def _hi_lo(x):
    import ml_dtypes
    bf = ml_dtypes.bfloat16
    hi = x.astype(np.float32).astype(bf)
    lo = (x.astype(np.float32) - hi.astype(np.float32)).astype(bf)
    return np.ascontiguousarray(hi), np.ascontiguousarray(lo)


def _prep_inputs(state, W1, b1, W2, b2, W3, b3):
    import ml_dtypes
    bf = ml_dtypes.bfloat16
    f = np.float32
    w1h, w1l = _hi_lo((W1 / 2).T)                                    # [S, H]
    w2h, w2l = _hi_lo((W2 / 4).T)                                    # [H, H]
    # layer-3 weights: 2^(t-18)*W3, hi/lo packed along M: [k, t*16 + (hi8|lo8)]
    w3pack = np.zeros((H, T * 16), dtype=bf)
    for t in range(1, T + 1):
        wt = ((2.0 ** (t - 18)) * W3.T).astype(f)                    # [H, A]
        hi = wt.astype(bf)
        lo = (wt - hi.astype(f)).astype(bf)
        w3pack[:, (t - 1) * 16:(t - 1) * 16 + 8] = hi
        w3pack[:, (t - 1) * 16 + 8:t * 16] = lo
    b1h = np.ascontiguousarray((b1 / 2).reshape(2, 128), dtype=f)
    b2q = (0.25 * W2.sum(axis=1) + 0.5 * b2).astype(f)               # [H]
    b2thr = np.zeros((2, 128, T), dtype=f)
    b2add = np.zeros((2, 128, T), dtype=f)
    b2cols = np.zeros((2, 128, 2), dtype=f)
    for m in range(2):
        bq = b2q[m * 128:(m + 1) * 128]
        for t in range(1, T + 1):
            b2thr[m, :, t - 1] = (2.0 ** (t - 1)) * (1.0 - bq)
            b2add[m, :, t - 1] = (2.0 ** (t - 1)) * bq
        b2cols[m, :, 0] = -bq          # t=0 state init (bias-shifted)
        b2cols[m, :, 1] = bq - 1.0     # Sign bias
    b3z = np.zeros((16, 1), dtype=f)
    b3z[:A, 0] = (1 - 2.0 ** -16) * (0.5 * W3.sum(axis=1) + b3)
    ident = np.eye(128, dtype=f)
    shards = state.reshape(NCORES, B_C, S)
    in_maps = []
    for i in range(NCORES):
        in_maps.append({
            "state": np.ascontiguousarray(shards[i], dtype=f),
            "w1ht0": w1h, "w1ht1": w1l, "w2qt0": w2h, "w2qt1": w2l,
            "w3pack": np.ascontiguousarray(w3pack),
            "b1h": b1h, "b2thr": b2thr, "b2add": b2add, "b2cols": b2cols,
            "b3z": b3z, "ident": ident,
        })
    return in_maps


def _patch_ldw_opt():
    """Enable walrus ldweights dedup (consecutive same-weight matmuls)."""
    if _CACHE.get("ldw_patched"):
        return
    import concourse.bass_utils as bu
    orig = bu.run_command

    def patched(argv, **kw):
        argv = [a
                for a in argv]
        return orig(argv, **kw)

    bu.run_command = patched
    _CACHE["ldw_patched"] = True


def _get_nc():
    if "nc" not in _CACHE:
        _patch_ldw_opt()
        nc = _build_nc()
        nc.finalize()
        _CACHE["nc"] = nc
    return _CACHE["nc"]


def kernel(state, W1, b1, W2, b2, W3, b3, _trace=False, _trace_kwargs=None):
    from concourse.bass_utils import run_bass_kernel_spmd

    nc = _get_nc()
    in_maps = _prep_inputs(state, W1, b1, W2, b2, W3, b3)
    res = run_bass_kernel_spmd(
        nc, in_maps, core_ids=list(range(NCORES)),
        trace=_trace, **(_trace_kwargs or {}))
    outs = [res.results[i]["out"] for i in range(NCORES)]
    full = np.concatenate([np.asarray(o).reshape(B_C, A) for o in outs], axis=0)
    if _trace:
        return full.astype(np.float32), res
    return full.astype(np.float32)


# revision 26
# speedup vs baseline: 1.0337x; 1.0337x over previous
"""Trainium2 Bass kernel for the spiking-actor MLP (nn_Actor_3504693313593).

Math (per batch element, T=16 steps, LIF tau=2, v_th=1, hard reset to 0):
  h1 = state @ W1.T + b1                      (loop invariant)
  per step: v1 = (v1 + h1)/2; s1 = v1>=1; v1 *= 1-s1
            h2 = s1 @ W2.T + b2; v2 = (v2 + h2)/2; s2 = v2>=1; v2 *= 1-s2
            v3 = (v3 + s2 @ W3.T + b3)/2      (non-spiking, linear)
  out = tanh(v3)

Device formulation (validated exact vs reference):
  - spikes carried as g = 2s-1 in {-1,+1} (ScalarE Sign), the +1 halves folded
    into weights/biases on the host.
  - per-layer LIF state kept TIME-SCALED and BIAS-SHIFTED:  Wt = 2^t (u_t - b)
    so one fused custom-DVE op per layer per step does charge+reset+bias:
        Wt' = select(Wt < 2^(t-1)(1-b), Wt + 2^(t-1) b, 0) + mm * 2^t
    (for layer 1 the bias lives inside h1h, so the C1 slot is 0).
  - spikes: g = Sign(2^-t * Wt + (b-1)) on ScalarE with per-partition bias.
  - layer 3 is linear in s2 -> collapsed into a PSUM accumulation over steps
    with host-prescaled weights 2^(t-18)*W3, hi/lo packed along M (16 rows),
    4 batch-chunks at PSUM col-groups 0..3 of a single bank.
  - matmuls in bf16 with hi/lo splitting (exact to 2^-17); spikes exact bf16.
Sharding: pure data parallel, batch/8 per NeuronCore, weights replicated.
"""

import numpy as np

S, H, A, T = 128, 256, 8, 16
NCORES = 8
B_GLOBAL = 131072
B_C = B_GLOBAL // NCORES   # 16384 rows per core
N_B = 2048                 # batch-tile (SBUF free dim)
NT = B_C // N_B            # 8 batch tiles per core
CH = 512                   # matmul moving-dim chunk
NCH = N_B // CH            # 4 chunks per tile
HF = N_B // 2              # 1024: psum tile free dim (2 banks)
NBLK = N_B // 128          # 16 128-row blocks per tile

_CACHE = {}


def _register_lif_op():
    """Fused LIF step: out = select(in1 < s0, in1 + s1, 0) + in0*imm2."""
    import concourse.dve_ops as dve_ops
    if "LIF_STEP_ANT" in dve_ops._SUB_OPCODE_FOR_NAME:
        return next(o for o in dve_ops.OPS if o.name == "LIF_STEP_ANT")
    from concourse.dve_spec import Spec, Src0, Src1, C0, C1, C2, Zero, select, lower
    from concourse.dve_uop import DveOpSpec

    body = select(Src1 < C0, Src1 + C1, Zero) + Src0 * C2
    spec = Spec(
        body=body,
        reference=lambda in0, in1, s0, s1, imm2: (
            np.where(in1 < s0, in1 + s1, 0.0) + in0.astype(np.float32) * imm2
        ).astype(np.float32),
    )
    row = max(dve_ops._SUB_OPCODE_FOR_NAME.values()) + 1
    dve_ops._SUB_OPCODE_FOR_NAME["LIF_STEP_ANT"] = row
    shas = {}
    for ver in ("v3",):
        tmp = DveOpSpec(name="LIF_STEP_ANT", opcode=row,
                        uops=lower(spec, ver=ver), rd1_en=True)
        shas[ver] = tmp.sha(ver)
    op = dve_ops.DveOp("LIF_STEP_ANT", spec, subdim=False, uops_sha=shas)
    dve_ops.OPS.append(op)
    dve_ops.CUSTOM_DVE_SPECS[op.name] = spec
    return op


def _build_nc():
    import concourse.bass as bass
    import concourse.bacc as bacc
    import concourse.mybir as mybir

    f32 = mybir.dt.float32
    bf16 = mybir.dt.bfloat16
    AF = mybir.ActivationFunctionType
    OP = mybir.AluOpType
    from concourse.tile import TileContext

    LIF = _register_lif_op()

    nc = bacc.Bacc()
    state_d = nc.declare_dram_parameter("state", [B_C, S], f32, isOutput=False)
    w1_d = [nc.declare_dram_parameter(f"w1ht{h}", [S, H], bf16, isOutput=False)
            for h in range(2)]                                    # (W1/2).T hi/lo
    w2_d = [nc.declare_dram_parameter(f"w2qt{h}", [H, H], bf16, isOutput=False)
            for h in range(2)]                                    # (W2/4).T hi/lo
    w3_d = nc.declare_dram_parameter("w3pack", [H, T * 16], bf16, isOutput=False)
    b1_d = nc.declare_dram_parameter("b1h", [2, 128], f32, isOutput=False)
    bthr_d = nc.declare_dram_parameter("b2thr", [2, 128, T], f32, isOutput=False)
    badd_d = nc.declare_dram_parameter("b2add", [2, 128, T], f32, isOutput=False)
    bcol_d = nc.declare_dram_parameter("b2cols", [2, 128, 2], f32, isOutput=False)
    b3_d = nc.declare_dram_parameter("b3z", [16, 1], f32, isOutput=False)
    id_d = nc.declare_dram_parameter("ident", [128, 128], f32, isOutput=False)
    out_d = nc.declare_dram_parameter("out", [B_C, A], f32, isOutput=True)

    with TileContext(nc) as tc:
        with (
            tc.tile_pool(name="wp", bufs=1) as wp,
            tc.tile_pool(name="stin", bufs=2) as stin_p,
            tc.tile_pool(name="stT", bufs=2) as stT_p,
            tc.tile_pool(name="h1h", bufs=2) as h1h_p,
            tc.tile_pool(name="st8", bufs=1) as st_p,
            tc.tile_pool(name="y3", bufs=4) as y3_p,
            tc.tile_pool(name="outT", bufs=2) as outT_p,
            tc.tile_pool(name="ps", bufs=3, space="PSUM") as ps_p,
            tc.tile_pool(name="psv3", bufs=1, space="PSUM") as psv3_p,
        ):
            # ---- load weights/constants (once) ----
            w1t = [wp.tile([S, H], bf16, tag=f"w1t{h}", name=f"w1t{h}")
                   for h in range(2)]
            for h in range(2):
                nc.sync.dma_start(w1t[h][:], w1_d[h][:])
            w2t = [[wp.tile([128, H], bf16, tag=f"w2t{h}{k}", name=f"w2t{h}{k}")
                    for k in range(2)] for h in range(2)]
            for h in range(2):
                for k in range(2):
                    nc.sync.dma_start(w2t[h][k][:], w2_d[h][k * 128:(k + 1) * 128, :])
            w3t = [wp.tile([128, T * 16], bf16, tag=f"w3p{k}", name=f"w3p{k}")
                   for k in range(2)]
            for k in range(2):
                nc.sync.dma_start(w3t[k][:], w3_d[k * 128:(k + 1) * 128, :])
            b1t = wp.tile([128, 2], f32, tag="b1t", name="b1t")
            nc.sync.dma_start(b1t[:], b1_d[:].rearrange("m p -> p m"))
            bthr = [wp.tile([128, T], f32, tag=f"bthr{m}", name=f"bthr{m}")
                    for m in range(2)]
            badd = [wp.tile([128, T], f32, tag=f"badd{m}", name=f"badd{m}")
                    for m in range(2)]
            bcol = [wp.tile([128, 2], f32, tag=f"bcol{m}", name=f"bcol{m}")
                    for m in range(2)]
            for m in range(2):
                nc.sync.dma_start(bthr[m][:], bthr_d[m])
                nc.sync.dma_start(badd[m][:], badd_d[m])
                nc.sync.dma_start(bcol[m][:], bcol_d[m])
            b3t16 = wp.tile([16, 1], f32, tag="b3t16", name="b3t16")
            nc.sync.dma_start(b3t16[:], b3_d[:])
            idt = wp.tile([128, 128], f32, tag="idt", name="idt")
            nc.sync.dma_start(idt[:], id_d[:])
            neg1 = wp.tile([128, 1], f32, tag="neg1", name="neg1")
            nc.vector.memset(neg1[:], -1.0)

            def load_phase(bt):
                """DMA state tile, transpose to [S, N_B] hi/lo, h1h matmuls."""
                t0 = bt * N_B
                st_in = stin_p.tile([128, N_B], f32, tag="st_in", name="st_in")
                nc.sync.dma_start(
                    st_in[:].rearrange("p (c s) -> p c s", s=S),
                    state_d[t0:t0 + N_B, :].rearrange("(c p) s -> p c s", p=128),
                )
                stTh = stT_p.tile([S, N_B], bf16, tag="stTh", name="stTh")
                stTl = stT_p.tile([S, N_B], bf16, tag="stTl", name="stTl")
                for c in range(NBLK):
                    pst = ps_p.tile([128, 128], f32, tag="ps_mm", name="ps_tr")
                    nc.tensor.transpose(
                        pst[:], st_in[:, c * 128:(c + 1) * 128], idt[:])
                    sl = slice(c * 128, (c + 1) * 128)
                    nc.scalar.activation(stTh[:, sl], pst[:], AF.Copy)
                    nc.vector.tensor_tensor(
                        stTl[:, sl], pst[:], stTh[:, sl], OP.subtract)
                h1h = [h1h_p.tile([128, N_B], f32, tag=f"h1h{m}", name=f"h1h{m}")
                       for m in range(2)]
                for m in range(2):
                    msl = slice(m * 128, (m + 1) * 128)
                    for c in range(NCH):
                        psm = ps_p.tile([128, CH], f32, tag="ps_mm", name="ps_mm")
                        csl = slice(c * CH, (c + 1) * CH)
                        nc.tensor.matmul(
                            psm[:, :CH], w1t[0][:, msl], stTh[:, csl],
                            start=True, stop=False)
                        nc.tensor.matmul(
                            psm[:, :CH], w1t[0][:, msl], stTl[:, csl],
                            start=False, stop=False)
                        nc.tensor.matmul(
                            psm[:, :CH], w1t[1][:, msl], stTh[:, csl],
                            start=False, stop=True)
                        nc.scalar.activation(
                            h1h[m][:, csl], psm[:, :CH],
                            AF.Identity, bias=b1t[:, m:m + 1])
                return h1h

            h1h_next = load_phase(0)
            for bt in range(NT):
                t0 = bt * N_B
                h1h = h1h_next
                # ---- states (time-scaled, bias-shifted) + spike tiles ----
                u1 = [st_p.tile([128, N_B], f32, tag=f"u1_{m}", name=f"u1_{m}")
                      for m in range(2)]
                u2 = [st_p.tile([128, N_B], f32, tag=f"u2_{m}", name=f"u2_{m}")
                      for m in range(2)]
                g1 = [[st_p.tile([128, N_B], bf16, tag=f"g1_{d}{m}",
                                 name=f"g1_{d}{m}") for m in range(2)]
                      for d in range(2)]
                g2 = [[st_p.tile([128, N_B], bf16, tag=f"g2_{d}{m}",
                                 name=f"g2_{d}{m}") for m in range(2)]
                      for d in range(2)]
                for m in range(2):
                    nc.vector.memset(u1[m][:], 0.0)
                    # u2_0 = -b2q (bias-shifted state at t=0)
                    nc.vector.tensor_copy(
                        u2[m][:], bcol[m][:, 0:1].to_broadcast([128, N_B]))
                pv3 = psv3_p.tile([128, CH], f32, tag="pv3", name="pv3")

                for t in range(1, T + 1):
                    th = float(2.0 ** (t - 1))
                    sc = float(2.0 ** t)
                    # ---- layer 1: fused LIF (bias inside h1h) + Sign ----
                    for m in range(2):
                        nc.vector._custom_dve(
                            LIF, out=u1[m][:], in0=h1h[m][:], in1=u1[m][:],
                            s0=th, s1=0.0, imm2=sc)
                    for m in range(2):
                        for j in range(2):
                            jsl = slice(j * HF, (j + 1) * HF)
                            nc.scalar.activation(
                                g1[t % 2][m][:, jsl], u1[m][:, jsl], AF.Sign,
                                bias=neg1[:, 0:1], scale=1.0 / sc)
                    # ---- layer 2: mm in psum, fused LIF from psum + Sign ----
                    for m in range(2):
                        msl = slice(m * 128, (m + 1) * 128)
                        pj = [ps_p.tile([128, HF], f32, tag="ps_mm", name="pmm")
                              for _ in range(2)]
                        for hk, (hh, kk) in enumerate(
                                ((0, 0), (0, 1), (1, 0), (1, 1))):
                            for j in range(2):
                                for cc in range(2):
                                    csl = slice((j * 2 + cc) * CH,
                                                (j * 2 + cc + 1) * CH)
                                    nc.tensor.matmul(
                                        pj[j][:, cc * CH:(cc + 1) * CH],
                                        w2t[hh][kk][:, msl],
                                        g1[t % 2][kk][:, csl],
                                        start=(hk == 0), stop=(hk == 3))
                        for j in range(2):
                            jsl = slice(j * HF, (j + 1) * HF)
                            nc.vector._custom_dve(
                                LIF, out=u2[m][:, jsl], in0=pj[j][:],
                                in1=u2[m][:, jsl],
                                s0=bthr[m][:, t - 1:t], s1=badd[m][:, t - 1:t],
                                imm2=sc)
                    for m in range(2):
                        for j in range(2):
                            jsl = slice(j * HF, (j + 1) * HF)
                            nc.scalar.activation(
                                g2[t % 2][m][:, jsl], u2[m][:, jsl], AF.Sign,
                                bias=bcol[m][:, 1:2], scale=1.0 / sc)
                    # ---- layer 3: pv3[32c:32c+16] += W3pack_t @ g2 chunks ----
                    for c in range(NCH):
                        for k in range(2):
                            nc.tensor.matmul(
                                pv3[32 * c:32 * c + 16, :],
                                w3t[k][:, (t - 1) * 16:t * 16],
                                g2[t % 2][k][:, c * CH:(c + 1) * CH],
                                start=(t == 1 and k == 0),
                                stop=(t == T and k == 1),
                                tile_position=(0, 32 * c))


                # ---- v3: evac [16,512] (+b3z on hi rows), transpose
                #      [16,128] blocks -> [128,16], hi+lo add on free dim,
                #      tanh, DMA out ----
                y3 = [y3_p.tile([16, CH], f32, tag=f"y3_{c}", name=f"y3_{c}")
                      for c in range(NCH)]
                for c in range(NCH):
                    nc.scalar.activation(
                        y3[c][:], pv3[32 * c:32 * c + 16, :], AF.Identity,
                        bias=b3t16[:, 0:1])
                outT = outT_p.tile([128, NBLK * A], f32, tag="outT", name="outT")
                for c in range(NCH):
                    for cc in range(4):
                        q = c * 4 + cc
                        pso = ps_p.tile([128, 16], f32, tag="ps_mm", name="ps_out")
                        nc.tensor.transpose(
                            pso[:, :16], y3[c][:, cc * 128:(cc + 1) * 128],
                            idt[:16, :16])
                        ev16 = y3_p.tile([128, 16], f32, tag="ev16", name="ev16")
                        nc.scalar.activation(ev16[:], pso[:, :16], AF.Copy)
                        nc.vector.tensor_tensor(
                            outT[:, q * A:(q + 1) * A],
                            ev16[:, 0:A], ev16[:, A:16], OP.add)
                nc.scalar.activation(outT[:], outT[:], AF.Tanh)
                nc.sync.dma_start(
                    out_d[t0:t0 + N_B, :].rearrange("(q p) a -> p q a", p=128),
                    outT[:].rearrange("p (q a) -> p q a", a=A),
                )
                if bt + 1 < NT:
                    h1h_next = load_phase(bt + 1)
    return nc


def _hi_lo(x):
    import ml_dtypes
    bf = ml_dtypes.bfloat16
    hi = x.astype(np.float32).astype(bf)
    lo = (x.astype(np.float32) - hi.astype(np.float32)).astype(bf)
    return np.ascontiguousarray(hi), np.ascontiguousarray(lo)


def _prep_inputs(state, W1, b1, W2, b2, W3, b3):
    import ml_dtypes
    bf = ml_dtypes.bfloat16
    f = np.float32
    w1h, w1l = _hi_lo((W1 / 2).T)                                    # [S, H]
    w2h, w2l = _hi_lo((W2 / 4).T)                                    # [H, H]
    # layer-3 weights: 2^(t-18)*W3, hi/lo packed along M: [k, t*16 + (hi8|lo8)]
    w3pack = np.zeros((H, T * 16), dtype=bf)
    for t in range(1, T + 1):
        wt = ((2.0 ** (t - 18)) * W3.T).astype(f)                    # [H, A]
        hi = wt.astype(bf)
        lo = (wt - hi.astype(f)).astype(bf)
        w3pack[:, (t - 1) * 16:(t - 1) * 16 + 8] = hi
        w3pack[:, (t - 1) * 16 + 8:t * 16] = lo
    b1h = np.ascontiguousarray((b1 / 2).reshape(2, 128), dtype=f)
    b2q = (0.25 * W2.sum(axis=1) + 0.5 * b2).astype(f)               # [H]
    b2thr = np.zeros((2, 128, T), dtype=f)
    b2add = np.zeros((2, 128, T), dtype=f)
    b2cols = np.zeros((2, 128, 2), dtype=f)
    for m in range(2):
        bq = b2q[m * 128:(m + 1) * 128]
        for t in range(1, T + 1):
            b2thr[m, :, t - 1] = (2.0 ** (t - 1)) * (1.0 - bq)
            b2add[m, :, t - 1] = (2.0 ** (t - 1)) * bq
        b2cols[m, :, 0] = -bq          # t=0 state init (bias-shifted)
        b2cols[m, :, 1] = bq - 1.0     # Sign bias
    b3z = np.zeros((16, 1), dtype=f)
    b3z[:A, 0] = (1 - 2.0 ** -16) * (0.5 * W3.sum(axis=1) + b3)
    ident = np.eye(128, dtype=f)
    shards = state.reshape(NCORES, B_C, S)
    in_maps = []
    for i in range(NCORES):
        in_maps.append({
            "state": np.ascontiguousarray(shards[i], dtype=f),
            "w1ht0": w1h, "w1ht1": w1l, "w2qt0": w2h, "w2qt1": w2l,
            "w3pack": np.ascontiguousarray(w3pack),
            "b1h": b1h, "b2thr": b2thr, "b2add": b2add, "b2cols": b2cols,
            "b3z": b3z, "ident": ident,
        })
    return in_maps


def _patch_ldw_opt():
    """Enable walrus ldweights dedup (consecutive same-weight matmuls)."""
    if _CACHE.get("ldw_patched"):
        return
    import concourse.bass_utils as bu
    orig = bu.run_command

    def patched(argv, **kw):
        argv = [a
                for a in argv]
        return orig(argv, **kw)

    bu.run_command = patched
    _CACHE["ldw_patched"] = True


def _get_nc():
    if "nc" not in _CACHE:
        _patch_ldw_opt()
        nc = _build_nc()
        nc.finalize()
        _CACHE["nc"] = nc
    return _CACHE["nc"]


def kernel(state, W1, b1, W2, b2, W3, b3, _trace=False, _trace_kwargs=None):
    from concourse.bass_utils import run_bass_kernel_spmd

    nc = _get_nc()
    in_maps = _prep_inputs(state, W1, b1, W2, b2, W3, b3)
    res = run_bass_kernel_spmd(
        nc, in_maps, core_ids=list(range(NCORES)),
        trace=_trace, **(_trace_kwargs or {}))
    outs = [res.results[i]["out"] for i in range(NCORES)]
    full = np.concatenate([np.asarray(o).reshape(B_C, A) for o in outs], axis=0)
    if _trace:
        return full.astype(np.float32), res
    return full.astype(np.float32)


# revision 28
# speedup vs baseline: 1.0464x; 1.0124x over previous
"""Trainium2 Bass kernel for the spiking-actor MLP (nn_Actor_3504693313593).

Math (per batch element, T=16 steps, LIF tau=2, v_th=1, hard reset to 0):
  h1 = state @ W1.T + b1                      (loop invariant)
  per step: v1 = (v1 + h1)/2; s1 = v1>=1; v1 *= 1-s1
            h2 = s1 @ W2.T + b2; v2 = (v2 + h2)/2; s2 = v2>=1; v2 *= 1-s2
            v3 = (v3 + s2 @ W3.T + b3)/2      (non-spiking, linear)
  out = tanh(v3)

Device formulation (validated exact vs reference):
  - spikes carried as g = 2s-1 in {-1,+1} (ScalarE Sign), the +1 halves folded
    into weights/biases on the host.
  - per-layer LIF state kept TIME-SCALED and BIAS-SHIFTED:  Wt = 2^t (u_t - b)
    so one fused custom-DVE op per layer per step does charge+reset+bias:
        Wt' = select(Wt < 2^(t-1)(1-b), Wt + 2^(t-1) b, 0) + mm * 2^t
    (for layer 1 the bias lives inside h1h, so the C1 slot is 0).
  - spikes: g = Sign(2^-t * Wt + (b-1)) on ScalarE with per-partition bias.
  - layer 3 is linear in s2 -> collapsed into a PSUM accumulation over steps
    with host-prescaled weights 2^(t-18)*W3, hi/lo packed along M (16 rows),
    4 batch-chunks at PSUM col-groups 0..3 of a single bank.
  - matmuls in bf16 with hi/lo splitting (exact to 2^-17); spikes exact bf16.
Sharding: pure data parallel, batch/8 per NeuronCore, weights replicated.
"""

import numpy as np

S, H, A, T = 128, 256, 8, 16
NCORES = 8
B_GLOBAL = 131072
B_C = B_GLOBAL // NCORES   # 16384 rows per core
N_B = 2048                 # batch-tile (SBUF free dim)
NT = B_C // N_B            # 8 batch tiles per core
CH = 512                   # matmul moving-dim chunk
NCH = N_B // CH            # 4 chunks per tile
HF = N_B // 2              # 1024: psum tile free dim (2 banks)
NBLK = N_B // 128          # 16 128-row blocks per tile

_CACHE = {}


def _register_lif_op():
    """Fused LIF step: out = select(in1 < s0, in1 + s1, 0) + in0*imm2."""
    import concourse.dve_ops as dve_ops
    if "LIF_STEP_ANT" in dve_ops._SUB_OPCODE_FOR_NAME:
        return next(o for o in dve_ops.OPS if o.name == "LIF_STEP_ANT")
    from concourse.dve_spec import Spec, Src0, Src1, C0, C1, C2, Zero, select, lower
    from concourse.dve_uop import DveOpSpec

    body = select(Src1 < C0, Src1 + C1, Zero) + Src0 * C2
    spec = Spec(
        body=body,
        reference=lambda in0, in1, s0, s1, imm2: (
            np.where(in1 < s0, in1 + s1, 0.0) + in0.astype(np.float32) * imm2
        ).astype(np.float32),
    )
    row = max(dve_ops._SUB_OPCODE_FOR_NAME.values()) + 1
    dve_ops._SUB_OPCODE_FOR_NAME["LIF_STEP_ANT"] = row
    shas = {}
    for ver in ("v3",):
        tmp = DveOpSpec(name="LIF_STEP_ANT", opcode=row,
                        uops=lower(spec, ver=ver), rd1_en=True)
        shas[ver] = tmp.sha(ver)
    op = dve_ops.DveOp("LIF_STEP_ANT", spec, subdim=False, uops_sha=shas)
    dve_ops.OPS.append(op)
    dve_ops.CUSTOM_DVE_SPECS[op.name] = spec
    return op


def _build_nc():
    import concourse.bass as bass
    import concourse.bacc as bacc
    import concourse.mybir as mybir

    f32 = mybir.dt.float32
    bf16 = mybir.dt.bfloat16
    AF = mybir.ActivationFunctionType
    OP = mybir.AluOpType
    from concourse.tile import TileContext

    LIF = _register_lif_op()

    nc = bacc.Bacc()
    state_d = nc.declare_dram_parameter("state", [B_C, S], f32, isOutput=False)
    w1_d = [nc.declare_dram_parameter(f"w1ht{h}", [S, H], bf16, isOutput=False)
            for h in range(2)]                                    # (W1/2).T hi/lo
    w2_d = [nc.declare_dram_parameter(f"w2qt{h}", [H, H], bf16, isOutput=False)
            for h in range(2)]                                    # (W2/4).T hi/lo
    w3_d = nc.declare_dram_parameter("w3pack", [H, T * 16], bf16, isOutput=False)
    b1_d = nc.declare_dram_parameter("b1h", [2, 128], f32, isOutput=False)
    bthr_d = nc.declare_dram_parameter("b2thr", [2, 128, T], f32, isOutput=False)
    badd_d = nc.declare_dram_parameter("b2add", [2, 128, T], f32, isOutput=False)
    bcol_d = nc.declare_dram_parameter("b2cols", [2, 128, 2], f32, isOutput=False)
    b3_d = nc.declare_dram_parameter("b3z", [16, 1], f32, isOutput=False)
    id_d = nc.declare_dram_parameter("ident", [128, 128], f32, isOutput=False)
    out_d = nc.declare_dram_parameter("out", [B_C, A], f32, isOutput=True)

    with TileContext(nc) as tc:
        with (
            tc.tile_pool(name="wp", bufs=1) as wp,
            tc.tile_pool(name="stin", bufs=2) as stin_p,
            tc.tile_pool(name="stT", bufs=2) as stT_p,
            tc.tile_pool(name="h1h", bufs=2) as h1h_p,
            tc.tile_pool(name="st8", bufs=1) as st_p,
            tc.tile_pool(name="y3", bufs=4) as y3_p,
            tc.tile_pool(name="outT", bufs=2) as outT_p,
            tc.tile_pool(name="ps", bufs=3, space="PSUM") as ps_p,
            tc.tile_pool(name="psv3", bufs=1, space="PSUM") as psv3_p,
        ):
            # ---- load weights/constants (once) ----
            w1t = [wp.tile([S, H], bf16, tag=f"w1t{h}", name=f"w1t{h}")
                   for h in range(2)]
            for h in range(2):
                nc.sync.dma_start(w1t[h][:], w1_d[h][:])
            w2t = [[wp.tile([128, H], bf16, tag=f"w2t{h}{k}", name=f"w2t{h}{k}")
                    for k in range(2)] for h in range(2)]
            for h in range(2):
                for k in range(2):
                    nc.sync.dma_start(w2t[h][k][:], w2_d[h][k * 128:(k + 1) * 128, :])
            w3t = [wp.tile([128, T * 16], bf16, tag=f"w3p{k}", name=f"w3p{k}")
                   for k in range(2)]
            for k in range(2):
                nc.sync.dma_start(w3t[k][:], w3_d[k * 128:(k + 1) * 128, :])
            b1t = wp.tile([128, 2], f32, tag="b1t", name="b1t")
            nc.sync.dma_start(b1t[:], b1_d[:].rearrange("m p -> p m"))
            bthr = [wp.tile([128, T], f32, tag=f"bthr{m}", name=f"bthr{m}")
                    for m in range(2)]
            badd = [wp.tile([128, T], f32, tag=f"badd{m}", name=f"badd{m}")
                    for m in range(2)]
            bcol = [wp.tile([128, 2], f32, tag=f"bcol{m}", name=f"bcol{m}")
                    for m in range(2)]
            for m in range(2):
                nc.sync.dma_start(bthr[m][:], bthr_d[m])
                nc.sync.dma_start(badd[m][:], badd_d[m])
                nc.sync.dma_start(bcol[m][:], bcol_d[m])
            b3t16 = wp.tile([16, 1], f32, tag="b3t16", name="b3t16")
            nc.sync.dma_start(b3t16[:], b3_d[:])
            idt = wp.tile([128, 128], f32, tag="idt", name="idt")
            nc.sync.dma_start(idt[:], id_d[:])
            neg1 = wp.tile([128, 1], f32, tag="neg1", name="neg1")
            nc.vector.memset(neg1[:], -1.0)

            def load_phase(bt):
                """DMA state tile, transpose to [S, N_B] hi/lo, h1h matmuls."""
                t0 = bt * N_B
                st_in = stin_p.tile([128, N_B], f32, tag="st_in", name="st_in")
                nc.sync.dma_start(
                    st_in[:].rearrange("p (c s) -> p c s", s=S),
                    state_d[t0:t0 + N_B, :].rearrange("(c p) s -> p c s", p=128),
                )
                stTh = stT_p.tile([S, N_B], bf16, tag="stTh", name="stTh")
                stTl = stT_p.tile([S, N_B], bf16, tag="stTl", name="stTl")
                for c in range(NBLK):
                    pst = ps_p.tile([128, 128], f32, tag="ps_mm", name="ps_tr")
                    nc.tensor.transpose(
                        pst[:], st_in[:, c * 128:(c + 1) * 128], idt[:])
                    sl = slice(c * 128, (c + 1) * 128)
                    nc.scalar.activation(stTh[:, sl], pst[:], AF.Copy)
                    nc.vector.tensor_tensor(
                        stTl[:, sl], pst[:], stTh[:, sl], OP.subtract)
                h1h = [h1h_p.tile([128, N_B], f32, tag=f"h1h{m}", name=f"h1h{m}")
                       for m in range(2)]
                for m in range(2):
                    msl = slice(m * 128, (m + 1) * 128)
                    for c in range(NCH):
                        psm = ps_p.tile([128, CH], f32, tag="ps_mm", name="ps_mm")
                        csl = slice(c * CH, (c + 1) * CH)
                        nc.tensor.matmul(
                            psm[:, :CH], w1t[0][:, msl], stTh[:, csl],
                            start=True, stop=False)
                        nc.tensor.matmul(
                            psm[:, :CH], w1t[0][:, msl], stTl[:, csl],
                            start=False, stop=False)
                        nc.tensor.matmul(
                            psm[:, :CH], w1t[1][:, msl], stTh[:, csl],
                            start=False, stop=True)
                        nc.scalar.activation(
                            h1h[m][:, csl], psm[:, :CH],
                            AF.Identity, bias=b1t[:, m:m + 1])
                return h1h

            h1h_next = load_phase(0)
            for bt in range(NT):
                t0 = bt * N_B
                h1h = h1h_next
                # ---- states (time-scaled, bias-shifted) + spike tiles ----
                u1 = [st_p.tile([128, N_B], f32, tag=f"u1_{m}", name=f"u1_{m}")
                      for m in range(2)]
                u2 = [st_p.tile([128, N_B], f32, tag=f"u2_{m}", name=f"u2_{m}")
                      for m in range(2)]
                g1 = [[st_p.tile([128, N_B], bf16, tag=f"g1_{d}{m}",
                                 name=f"g1_{d}{m}") for m in range(2)]
                      for d in range(2)]
                g2 = [[st_p.tile([128, N_B], bf16, tag=f"g2_{d}{m}",
                                 name=f"g2_{d}{m}") for m in range(2)]
                      for d in range(2)]
                pv3 = psv3_p.tile([128, CH], f32, tag="pv3", name="pv3")

                for t in range(1, T + 1):
                    th = float(2.0 ** (t - 2))
                    sc = float(2.0 ** (t - 1))
                    # ---- layer 1: fused LIF (bias inside h1h) + Sign ----
                    # state Wt = 2^(t-1)*u1_t; W1_1 = h1h (free alias)
                    u1c = h1h if t == 1 else u1
                    if t > 1:
                        for m in range(2):
                            nc.vector._custom_dve(
                                LIF, out=u1[m][:], in0=h1h[m][:],
                                in1=(h1h if t == 2 else u1)[m][:],
                                s0=th, s1=0.0, imm2=sc)
                    for m in range(2):
                        for j in range(2):
                            jsl = slice(j * HF, (j + 1) * HF)
                            nc.scalar.activation(
                                g1[t % 2][m][:, jsl], u1c[m][:, jsl], AF.Sign,
                                bias=neg1[:, 0:1], scale=1.0 / sc)
                    # ---- layer 2: mm in psum, fused LIF from psum + Sign ----
                    for m in range(2):
                        msl = slice(m * 128, (m + 1) * 128)
                        pj = [ps_p.tile([128, HF], f32, tag="ps_mm", name="pmm")
                              for _ in range(2)]
                        for hk, (hh, kk) in enumerate(
                                ((0, 0), (0, 1), (1, 0), (1, 1))):
                            for j in range(2):
                                for cc in range(2):
                                    csl = slice((j * 2 + cc) * CH,
                                                (j * 2 + cc + 1) * CH)
                                    nc.tensor.matmul(
                                        pj[j][:, cc * CH:(cc + 1) * CH],
                                        w2t[hh][kk][:, msl],
                                        g1[t % 2][kk][:, csl],
                                        start=(hk == 0), stop=(hk == 3))
                        for j in range(2):
                            jsl = slice(j * HF, (j + 1) * HF)
                            if t == 1:
                                # W2_1 = u2_1 - b2q = mm (scale 2^0)
                                nc.vector.tensor_copy(u2[m][:, jsl], pj[j][:])
                            else:
                                nc.vector._custom_dve(
                                    LIF, out=u2[m][:, jsl], in0=pj[j][:],
                                    in1=u2[m][:, jsl],
                                    s0=bthr[m][:, t - 2:t - 1],
                                    s1=badd[m][:, t - 2:t - 1],
                                    imm2=sc)
                    for m in range(2):
                        for j in range(2):
                            jsl = slice(j * HF, (j + 1) * HF)
                            nc.scalar.activation(
                                g2[t % 2][m][:, jsl], u2[m][:, jsl], AF.Sign,
                                bias=bcol[m][:, 1:2], scale=1.0 / sc)
                    # ---- layer 3: pv3[32c:32c+16] += W3pack_t @ g2 chunks ----
                    for c in range(NCH):
                        for k in range(2):
                            nc.tensor.matmul(
                                pv3[32 * c:32 * c + 16, :],
                                w3t[k][:, (t - 1) * 16:t * 16],
                                g2[t % 2][k][:, c * CH:(c + 1) * CH],
                                start=(t == 1 and k == 0),
                                stop=(t == T and k == 1),
                                tile_position=(0, 32 * c))


                # ---- v3: evac [16,512] (+b3z on hi rows), transpose
                #      [16,128] blocks -> [128,16], hi+lo add on free dim,
                #      tanh, DMA out ----
                y3 = [y3_p.tile([16, CH], f32, tag=f"y3_{c}", name=f"y3_{c}")
                      for c in range(NCH)]
                for c in range(NCH):
                    nc.scalar.activation(
                        y3[c][:], pv3[32 * c:32 * c + 16, :], AF.Identity,
                        bias=b3t16[:, 0:1])
                outT = outT_p.tile([128, NBLK * A], f32, tag="outT", name="outT")
                for c in range(NCH):
                    for cc in range(4):
                        q = c * 4 + cc
                        pso = ps_p.tile([128, 16], f32, tag="ps_mm", name="ps_out")
                        nc.tensor.transpose(
                            pso[:, :16], y3[c][:, cc * 128:(cc + 1) * 128],
                            idt[:16, :16])
                        ev16 = y3_p.tile([128, 16], f32, tag="ev16", name="ev16")
                        nc.scalar.activation(ev16[:], pso[:, :16], AF.Copy)
                        nc.vector.tensor_tensor(
                            outT[:, q * A:(q + 1) * A],
                            ev16[:, 0:A], ev16[:, A:16], OP.add)
                nc.scalar.activation(outT[:], outT[:], AF.Tanh)
                nc.sync.dma_start(
                    out_d[t0:t0 + N_B, :].rearrange("(q p) a -> p q a", p=128),
                    outT[:].rearrange("p (q a) -> p q a", a=A),
                )
                if bt + 1 < NT:
                    h1h_next = load_phase(bt + 1)
    return nc


def _hi_lo(x):
    import ml_dtypes
    bf = ml_dtypes.bfloat16
    hi = x.astype(np.float32).astype(bf)
    lo = (x.astype(np.float32) - hi.astype(np.float32)).astype(bf)
    return np.ascontiguousarray(hi), np.ascontiguousarray(lo)


def _prep_inputs(state, W1, b1, W2, b2, W3, b3):
    import ml_dtypes
    bf = ml_dtypes.bfloat16
    f = np.float32
    w1h, w1l = _hi_lo((W1 / 2).T)                                    # [S, H]
    w2h, w2l = _hi_lo((W2 / 4).T)                                    # [H, H]
    # layer-3 weights: 2^(t-18)*W3, hi/lo packed along M: [k, t*16 + (hi8|lo8)]
    w3pack = np.zeros((H, T * 16), dtype=bf)
    for t in range(1, T + 1):
        wt = ((2.0 ** (t - 18)) * W3.T).astype(f)                    # [H, A]
        hi = wt.astype(bf)
        lo = (wt - hi.astype(f)).astype(bf)
        w3pack[:, (t - 1) * 16:(t - 1) * 16 + 8] = hi
        w3pack[:, (t - 1) * 16 + 8:t * 16] = lo
    b1h = np.ascontiguousarray((b1 / 2).reshape(2, 128), dtype=f)
    b2q = (0.25 * W2.sum(axis=1) + 0.5 * b2).astype(f)               # [H]
    b2thr = np.zeros((2, 128, T), dtype=f)
    b2add = np.zeros((2, 128, T), dtype=f)
    b2cols = np.zeros((2, 128, 2), dtype=f)
    for m in range(2):
        bq = b2q[m * 128:(m + 1) * 128]
        for t in range(1, T + 1):
            b2thr[m, :, t - 1] = (2.0 ** (t - 1)) * (1.0 - bq)
            b2add[m, :, t - 1] = (2.0 ** (t - 1)) * bq
        b2cols[m, :, 0] = -bq          # t=0 state init (bias-shifted)
        b2cols[m, :, 1] = bq - 1.0     # Sign bias
    b3z = np.zeros((16, 1), dtype=f)
    b3z[:A, 0] = (1 - 2.0 ** -16) * (0.5 * W3.sum(axis=1) + b3)
    ident = np.eye(128, dtype=f)
    shards = state.reshape(NCORES, B_C, S)
    in_maps = []
    for i in range(NCORES):
        in_maps.append({
            "state": np.ascontiguousarray(shards[i], dtype=f),
            "w1ht0": w1h, "w1ht1": w1l, "w2qt0": w2h, "w2qt1": w2l,
            "w3pack": np.ascontiguousarray(w3pack),
            "b1h": b1h, "b2thr": b2thr, "b2add": b2add, "b2cols": b2cols,
            "b3z": b3z, "ident": ident,
        })
    return in_maps


def _get_nc():
    if "nc" not in _CACHE:
        nc = _build_nc()
        nc.finalize()
        _CACHE["nc"] = nc
    return _CACHE["nc"]


def kernel(state, W1, b1, W2, b2, W3, b3, _trace=False, _trace_kwargs=None):
    from concourse.bass_utils import run_bass_kernel_spmd

    nc = _get_nc()
    in_maps = _prep_inputs(state, W1, b1, W2, b2, W3, b3)
    res = run_bass_kernel_spmd(
        nc, in_maps, core_ids=list(range(NCORES)),
        trace=_trace, **(_trace_kwargs or {}))
    outs = [res.results[i]["out"] for i in range(NCORES)]
    full = np.concatenate([np.asarray(o).reshape(B_C, A) for o in outs], axis=0)
    if _trace:
        return full.astype(np.float32), res
    return full.astype(np.float32)
